# revision 45
# baseline (speedup 1.0000x reference)
"""GCN policy network on 8 TRN2 NeuronCores.

Design:
  - 64 graphs are padded to fixed 2048-node "slots" (virtual node space
    V = 131072); core c owns graphs [8c, 8c+8) = 16384 virtual nodes.
  - Edges are routed to the core that owns their destination (col) node.
  - Per layer: node features h are projected (x@W) locally per core,
    scaled by dis[row] (the 1/sqrt(deg) factor of the *source*), and
    AllGathered into a full per-core HBM table [V, 64].
  - Message passing per core: gpsimd.dma_gather fetches h'[row] rows
    (int16 indices, 4 buckets of 32768 rows), a small selection matrix
    S[e, 0:16] (built on DVE from precomputed col-rank + edge weight)
    folds the per-edge weight and segment-sum into one PE matmul per
    128-edge block, PE-transpose puts results in [dest, feat] layout and
    gpsimd.dma_scatter_add accumulates into an HBM accumulator, which
    absorbs all data-dependent destination offsets.
  - Readback scales by dis[col], builds BN stats (AllReduce), applies
    BN+ReLU (+ validity mask to kill padded nodes), then either the next
    projection or the per-graph max readout (AllGather) and the tiny
    speed/route/head MLPs (computed redundantly on every core).
"""

import sys

import numpy as np

if "/opt/trn_rl_repo" not in sys.path:
    sys.path.insert(0, "/opt/trn_rl_repo")

# problem constants
N = 100000
E = 1600000
B = 64
F = 128
H = 64
A = 5
RL = 10
EPS = 1e-5

NC = 8
GSLOT = 2048          # virtual nodes per graph slot
W = 16                # max distinct dests per 128-edge block
CALLNB = 8            # blocks per gather call (dma_gather caps at 1024 idxs)
CHUNKNB = 64          # blocks per scatter call (caps at 1024 entries)

LAST_EXEC_NS = None
_CACHE = {}


def _pack_core(vrow, vcol_loc, wgt, BUCK, NBUCK):
    """Order one core's edges (bucket asc, col asc), pack into blocks of
    <=128 edges covering <= W *whole* destinations (a dest's edges are
    never split across blocks, so scatter indices are unique per bucket).
    Returns per-block arrays."""
    buck = vrow // BUCK
    order = np.lexsort((vcol_loc, buck))
    vr = vrow[order]
    vc = vcol_loc[order]
    ww = wgt[order]
    bk = buck[order]
    bbounds = np.searchsorted(bk, np.arange(NBUCK + 1))
    blocks = []  # (start, end, bucket)
    for b in range(NBUCK):
        s0, e0 = int(bbounds[b]), int(bbounds[b + 1])
        if s0 == e0:
            continue
        seg = vc[s0:e0]
        # start index of each destination run within the segment
        runs = np.concatenate(
            [[0], np.flatnonzero(np.diff(seg)) + 1, [e0 - s0]]
        )
        rl = np.diff(runs)
        assert rl.max() <= 128, f"in-degree {rl.max()} > 128 in one bucket"
        i = 0
        nr = len(rl)
        while i < nr:
            ne = 0
            j = i
            while j < nr and j - i < W and ne + rl[j] <= 128:
                ne += rl[j]
                j += 1
            blocks.append((s0 + int(runs[i]), s0 + int(runs[j]), b))
            i = j
    return vr, vc, ww, blocks


def _preprocess(x, edge_index, edge_weight, batch_idx):
    """Host preprocessing: virtual node mapping, per-core edge packing,
    upload arrays. Returns (sched, percore dicts, postmeta)."""
    V = B * GSLOT
    VPC = V // NC
    NBUCK = 4
    BUCK = V // NBUCK
    CH = VPC // 128
    assert BUCK <= 32768

    bi = np.asarray(batch_idx).astype(np.int64)
    counts = np.bincount(bi, minlength=B)
    assert counts.max() <= GSLOT, f"graph too large: {counts.max()}"
    starts = np.zeros(B + 1, np.int64)
    np.cumsum(counts, out=starts[1:])
    # node i (sorted by graph) -> virtual node id
    vmap = (np.arange(len(bi)) - starts[bi] + bi * GSLOT).astype(np.int64)

    row = np.asarray(edge_index[0]).astype(np.int64)
    col = np.asarray(edge_index[1]).astype(np.int64)
    ew = np.asarray(edge_weight, np.float64)
    nreal = len(bi)

    deg = np.bincount(col, weights=ew, minlength=nreal) + 1.0
    dis = (1.0 / np.sqrt(deg)).astype(np.float32)

    # edge stream incl. self loops (weight 1)
    ar = np.concatenate([vmap[row], vmap])
    ac = np.concatenate([vmap[col], vmap])
    aw = np.concatenate([ew.astype(np.float32), np.ones(nreal, np.float32)])

    dis_v = np.zeros(V, np.float32)
    dis_v[vmap] = dis

    core_of = ac // VPC
    percore_raw = []
    nb_bucket = np.zeros((NC, NBUCK), np.int64)
    for c in range(NC):
        m = core_of == c
        vr, vc, ww, blocks = _pack_core(
            ar[m], (ac[m] - c * VPC), aw[m], BUCK, NBUCK
        )
        percore_raw.append((vr, vc, ww, blocks))
        for (s, e, b) in blocks:
            nb_bucket[c, b] += 1

    NBb = [int(-(-nb_bucket[:, b].max() // 8) * 8) for b in range(NBUCK)]
    NBtot = sum(NBb)

    # build uniform per-core arrays
    percore = []
    for c in range(NC):
        vr, vc, ww, blocks = percore_raw[c]
        idx_all = np.zeros((NBtot, 128), np.int16)
        crel_all = np.zeros((NBtot, 128), np.float32)
        norm_all = np.zeros((NBtot, 128), np.float32)
        dest_all = np.tile(
            (VPC + np.arange(W)).astype(np.int16)[None, :], (NBtot, 1)
        )
        kb = 0
        for b in range(NBUCK):
            blks = [blk for blk in blocks if blk[2] == b]
            for (s, e, _b) in blks:
                n = e - s
                rloc = vr[s:e] - b * BUCK
                assert rloc.min() >= 0 and rloc.max() < BUCK
                dl = np.unique(vc[s:e])  # sorted distinct dests (local)
                assert len(dl) <= W
                cr = np.searchsorted(dl, vc[s:e])
                idx_all[kb, :n] = rloc.astype(np.int16)
                crel_all[kb, :n] = cr.astype(np.float32)
                norm_all[kb, :n] = ww[s:e]
                dpad = (VPC + np.arange(W)).astype(np.int16)
                dpad[: len(dl)] = dl.astype(np.int16)
                dest_all[kb] = dpad
                kb += 1
            kb = sum(NBb[: b + 1])  # null blocks pad to NBb[b]
        # wrapped layouts
        gidx = np.tile(
            idx_all.reshape(NBtot * 8, 16).T, (8, 1)
        )  # [128, NBtot*8]
        crel = crel_all.T.copy()  # [128, NBtot]
        nrm = norm_all.T.copy()
        sidx = np.tile(dest_all.T, (8, 1))  # [128, NBtot]
        disv = (
            dis_v[c * VPC : (c + 1) * VPC].reshape(CH, 128).T.copy()
        )  # [128, CH]
        maskrow = (dis_v[c * VPC : (c + 1) * VPC] > 0).astype(np.float32)
        mask = np.tile(maskrow[None, :], (H, 1))  # [64, VPC]
        xt = np.zeros((F, VPC), np.float32)
        lo, hi = c * VPC, (c + 1) * VPC
        sel = (vmap >= lo) & (vmap < hi)
        xt[:, vmap[sel] - lo] = np.asarray(x, np.float32)[sel].T
        percore.append(
            dict(gidx=gidx, crel=crel, nrm=nrm, sidx=sidx, disv=disv,
                 mask=mask, xt=xt)
        )

    sched = dict(NBb=tuple(NBb), NBtot=NBtot, V=V, VPC=VPC, BUCK=BUCK,
                 NBUCK=NBUCK, CH=CH, NREAL=nreal)
    return sched, percore, vmap


def _chunks(sched):
    """Chunks of <= CHUNKNB blocks (one scatter each), never crossing a
    bucket boundary. Each chunk is processed in gather calls of CALLNB."""
    out = []
    blk0 = 0
    for b in range(sched["NBUCK"]):
        nb_left = sched["NBb"][b]
        while nb_left > 0:
            nb = min(CHUNKNB, nb_left)
            out.append((b, blk0, nb))
            blk0 += nb
            nb_left -= nb
    return out


def _build(sched):
    import os as _os

    import concourse.bass as bass
    import concourse.bacc as bacc
    import concourse.mybir as mybir
    from concourse import tile

    stage = int(_os.environ.get("KBSTAGE", "9"))
    sub = int(_os.environ.get("KBSUB", "9"))

    f32 = mybir.dt.float32
    i16 = mybir.dt.int16
    AO = mybir.AluOpType
    AF = mybir.ActivationFunctionType
    AX = mybir.AxisListType

    V = sched["V"]
    VPC = sched["VPC"]
    BUCK = sched["BUCK"]
    CH = sched["CH"]
    NBtot = sched["NBtot"]
    NREAL = sched["NREAL"]
    RG = [list(range(NC))]

    nc = bacc.Bacc(num_devices=NC)
    dp = nc.declare_dram_parameter
    xt_d = dp("xt", [F, VPC], f32, isOutput=False)
    w1_d = dp("w1", [F, H], f32, isOutput=False)
    w2_d = dp("w2", [H, H], f32, isOutput=False)
    eye_d = dp("eye", [128, 128], f32, isOutput=False)
    iota_d = dp("iota", [128, W], f32, isOutput=False)
    gidx_d = dp("gidx", [128, NBtot * 8], i16, isOutput=False)
    crel_d = dp("crel", [128, NBtot], f32, isOutput=False)
    nrm_d = dp("nrm", [128, NBtot], f32, isOutput=False)
    sidx_d = dp("sidx", [128, NBtot], i16, isOutput=False)
    mask_d = dp("mask", [H, VPC], f32, isOutput=False)
    disv_d = dp("disv", [128, CH], f32, isOutput=False)
    bnp_d = dp("bnp", [H, 4], f32, isOutput=False)
    swp_d = dp("swp", [1, 4], f32, isOutput=False)
    spt_d = dp("spt", [1, B], f32, isOutput=False)
    sgbe_d = dp("sgbe", [4, 2], f32, isOutput=False)
    cwf_d = dp("cwf", [6, 1], f32, isOutput=False)
    rim_d = dp("rim", [6, B * RL], f32, isOutput=False)
    rgbe_d = dp("rgbe", [1, 2], f32, isOutput=False)
    rw_d = dp("rw", [RL, 4], f32, isOutput=False)
    rbv_d = dp("rbv", [4, 1], f32, isOutput=False)
    ow1_d = dp("ow1", [H + 8, 16], f32, isOutput=False)
    ogbe_d = dp("ogbe", [16, 2], f32, isOutput=False)
    ow2_d = dp("ow2", [16, A], f32, isOutput=False)
    ob2_d = dp("ob2", [A, 1], f32, isOutput=False)
    out_d = dp("out", [A, B], f32, isOutput=True)

    dt_int = nc.dram_tensor
    h1sh_d = dt_int("h1sh", [VPC, H], f32, kind="Internal")
    h2sh_d = dt_int("h2sh", [VPC, H], f32, kind="Internal")
    tab1_d = dt_int("tab1", [V, H], f32, kind="Internal")
    tab2_d = dt_int("tab2", [V, H], f32, kind="Internal")
    acc1_d = dt_int("acc1", [VPC + 128, H], f32, kind="Internal")
    acc2_d = dt_int("acc2", [VPC + 128, H], f32, kind="Internal")
    st_in = [dt_int(f"st{l}in", [H, 8], f32, kind="Internal") for l in (1, 2)]
    st_out = [
        dt_int(f"st{l}out", [H, 8], f32, kind="Internal", addr_space="Shared")
        for l in (1, 2)
    ]
    gxsh_d = dt_int("gxsh", [H, NC], f32, kind="Internal")
    gxall_d = dt_int("gxall", [NC, H, NC], f32, kind="Internal",
                     addr_space="Shared")
    rcd_d = dt_int("rcd", [B * RL], f32, kind="Internal")

    chunks = _chunks(sched)

    with tile.TileContext(nc) as tc:
        with (
            tc.tile_pool(name="const", bufs=1) as cp,
            tc.tile_pool(name="big", bufs=1) as bigp,
        ):
            eye_t = cp.tile([128, 128], f32)
            nc.sync.dma_start(eye_t[:], eye_d[:])
            iota_t = cp.tile([128, W], f32)
            nc.sync.dma_start(iota_t[:], iota_d[:])
            w1_t = cp.tile([F, H], f32)
            nc.sync.dma_start(w1_t[:], w1_d[:])
            w2_t = cp.tile([H, H], f32)
            nc.sync.dma_start(w2_t[:], w2_d[:])
            dis_t = cp.tile([128, CH], f32)
            nc.sync.dma_start(dis_t[:], disv_d[:])
            bnp_t = cp.tile([H, 4], f32)
            nc.sync.dma_start(bnp_t[:], bnp_d[:])
            mask_t = bigp.tile([H, VPC], f32)
            nc.sync.dma_start(mask_t[:], mask_d[:])
            eps_t = cp.tile([128, 1], f32)
            nc.vector.memset(eps_t[:], EPS)
            zb_t = cp.tile([128, 1], f32)
            nc.vector.memset(zb_t[:], 0.0)

            # zero the two MP accumulators
            zfree = (VPC + 128) * H // 128
            zw = min(512, zfree)
            zt = cp.tile([128, zw], f32)
            nc.vector.memset(zt[:], 0.0)
            for acc in (acc1_d, acc2_d):
                av = acc[:].rearrange("(p a) f -> p (a f)", p=128)
                for k in range(zfree // zw):
                    nc.gpsimd.dma_start(av[:, k * zw:(k + 1) * zw], zt[:])
                rem = zfree - (zfree // zw) * zw
                if rem:
                    nc.gpsimd.dma_start(
                        av[:, zfree - rem:], zt[:, 0:rem]
                    )

            # ---- projection 1: h1 = (x @ W1) * dis  (node-major) ----
            with (
                tc.tile_pool(name="xp", bufs=3) as xp,
                tc.tile_pool(name="ps1", bufs=4, space="PSUM") as ps1,
                tc.tile_pool(name="hv", bufs=3) as hvp,
            ):
                for c in range(0 if _os.environ.get("KBNOPROJ") else CH):
                    xtt = xp.tile([128, 128], f32, tag="xtt")
                    nc.sync.dma_start(xtt[:], xt_d[:, c * 128:(c + 1) * 128])
                    p = ps1.tile([128, H], f32)
                    nc.tensor.matmul(p[:], xtt[:], w1_t[:], start=True,
                                     stop=True)
                    hr = hvp.tile([128, H], f32, tag="hr")
                    nc.vector.tensor_scalar(
                        hr[:], p[:], dis_t[:, c:c + 1], None, AO.mult
                    )
                    nc.sync.dma_start(h1sh_d[c * 128:(c + 1) * 128, :], hr[:])

            nc.gpsimd.collective_compute(
                "AllGather", AO.bypass, RG, [h1sh_d[:]], [tab1_d[:]]
            )

            layers = (1, 2) if stage >= 4 else ((1,) if stage >= 2 else ())
            for l in layers:
                tab = tab1_d if l == 1 else tab2_d
                acc = acc1_d if l == 1 else acc2_d
                mp_sb = bigp.tile([H, VPC], f32, tag="mp")

                # ---- message passing ----
                with (
                    tc.tile_pool(name=f"mp{l}", bufs=2) as mpp,
                    tc.tile_pool(name=f"pa{l}", bufs=4, space="PSUM") as pap,
                    tc.tile_pool(name=f"pb{l}", bufs=4, space="PSUM") as pbp,
                ):
                    for (b, blk0, cnb) in chunks:
                        cr = mpp.tile([128, cnb], f32, tag="cr")
                        nc.sync.dma_start(cr[:], crel_d[:, blk0:blk0 + cnb])
                        nm = mpp.tile([128, cnb], f32, tag="nm")
                        nc.sync.dma_start(nm[:], nrm_d[:, blk0:blk0 + cnb])
                        S_t = mpp.tile([128, cnb, W], f32, tag="S")
                        nc.vector.tensor_tensor(
                            S_t[:],
                            cr[:].unsqueeze(2).broadcast_to([128, cnb, W]),
                            iota_t[:].unsqueeze(1).broadcast_to(
                                [128, cnb, W]
                            ),
                            AO.is_equal,
                        )
                        nc.vector.tensor_tensor(
                            S_t[:], S_t[:],
                            nm[:].unsqueeze(2).broadcast_to([128, cnb, W]),
                            AO.mult,
                        )
                        scb = mpp.tile([128, cnb // 8, H], f32, tag="scb")
                        for g in range(cnb // 8):
                            kb0 = blk0 + g * 8
                            idx_t = mpp.tile([128, 64], i16, tag="idx")
                            nc.sync.dma_start(
                                idx_t[:], gidx_d[:, kb0 * 8:(kb0 + 8) * 8]
                            )
                            msg = mpp.tile([128, 8, H], f32, tag="msg")
                            nc.gpsimd.dma_gather(
                                msg[:], tab[b * BUCK:(b + 1) * BUCK, :],
                                idx_t[:], 1024, 1024, H,
                            )
                            pA = pap.tile([H, 128], f32)
                            for j in range(8):
                                nc.tensor.matmul(
                                    pA[:, j * W:(j + 1) * W],
                                    msg[:, j, :],
                                    S_t[:, g * 8 + j, :],
                                    start=True, stop=True,
                                )
                            t1 = mpp.tile([H, 128], f32, tag="t1")
                            nc.vector.tensor_copy(t1[:], pA[:])
                            pB = pbp.tile([128, H], f32)
                            nc.tensor.transpose(
                                pB[:], t1[:], eye_t[0:H, 0:H]
                            )
                            nc.vector.tensor_copy(scb[:, g, :], pB[:])
                        six = mpp.tile([128, cnb], i16, tag="six")
                        nc.sync.dma_start(six[:], sidx_d[:, blk0:blk0 + cnb])
                        nc.gpsimd.dma_scatter_add(
                            acc[:, :], scb[:], six[:], cnb * W, cnb * W, H
                        )

                if stage < 3:
                    continue
                # ---- readback + BN stats ----
                with (
                    tc.tile_pool(name=f"rb{l}", bufs=3) as rbp,
                    tc.tile_pool(name=f"pc{l}", bufs=4, space="PSUM") as pcp,
                    tc.tile_pool(name=f"sm{l}", bufs=1) as smp,
                ):
                    s1p = smp.tile([H, CH], f32, tag="s1p")
                    s2p = smp.tile([H, CH], f32, tag="s2p")
                    for c in range(CH):
                        rb = rbp.tile([128, H], f32, tag="rb")
                        nc.sync.dma_start(
                            rb[:], acc[c * 128:(c + 1) * 128, :]
                        )
                        nc.vector.tensor_scalar(
                            rb[:], rb[:], dis_t[:, c:c + 1], None, AO.mult
                        )
                        pC = pcp.tile([H, 128], f32)
                        nc.tensor.transpose(pC[:], rb[:], eye_t[:, :])
                        nc.vector.tensor_scalar(
                            mp_sb[:, c * 128:(c + 1) * 128], pC[:], 0.0,
                            0.0, AO.add, AO.add,
                            accum_out=s1p[:, c:c + 1],
                        )
                        sq = rbp.tile([H, 128], f32, tag="sq")
                        nc.scalar.activation(
                            sq[:], pC[:], AF.Square, bias=zb_t[0:H, :],
                            accum_out=s2p[:, c:c + 1],
                        )
                    st = smp.tile([H, 8], f32, tag="st")
                    nc.vector.memset(st[:], 0.0)
                    nc.vector.tensor_reduce(
                        st[:, 0:1], s1p[:], axis=AX.X, op=AO.add
                    )
                    nc.vector.tensor_reduce(
                        st[:, 1:2], s2p[:], axis=AX.X, op=AO.add
                    )
                    nc.sync.dma_start(st_in[l - 1][:], st[:])
                    nc.gpsimd.collective_compute(
                        "AllReduce", AO.add, RG,
                        [st_in[l - 1][:]], [st_out[l - 1][:]],
                    )
                    stf = smp.tile([H, 8], f32, tag="stf")
                    nc.sync.dma_start(stf[:], st_out[l - 1][:])
                    mean = smp.tile([H, 1], f32, tag="mean")
                    nc.vector.tensor_scalar(
                        mean[:], stf[:, 0:1], 1.0 / NREAL, None, AO.mult
                    )
                    var = smp.tile([H, 1], f32, tag="var")
                    # var = e2/n - mean^2
                    nc.vector.tensor_scalar(
                        var[:], stf[:, 1:2], 1.0 / NREAL, None, AO.mult
                    )
                    msq = smp.tile([H, 1], f32, tag="msq")
                    nc.vector.tensor_mul(msq[:], mean[:], mean[:])
                    nc.vector.tensor_sub(var[:], var[:], msq[:])
                    sd = smp.tile([H, 1], f32, tag="sd")
                    nc.scalar.activation(
                        sd[:], var[:], AF.Sqrt, bias=eps_t[0:H, :]
                    )
                    rinv = smp.tile([H, 1], f32, tag="rinv")
                    nc.vector.reciprocal(rinv[:], sd[:])
                    a_t = smp.tile([H, 1], f32, tag="a_t")
                    nc.vector.tensor_mul(
                        a_t[:], rinv[:], bnp_t[:, 2 * l - 2:2 * l - 1]
                    )
                    bsh = smp.tile([H, 1], f32, tag="bsh")
                    nc.vector.tensor_mul(msq[:], mean[:], a_t[:])
                    nc.vector.tensor_sub(
                        bsh[:], bnp_t[:, 2 * l - 1:2 * l], msq[:]
                    )
                    nc.scalar.activation(
                        mp_sb[:], mp_sb[:], AF.Relu, bias=bsh[:],
                        scale=a_t[:],
                    )
                    nc.vector.tensor_mul(mp_sb[:], mp_sb[:], mask_t[:])

                if stage < 4:
                    continue
                if l == 1:
                    # ---- projection 2 + table 2 ----
                    with (
                        tc.tile_pool(name="m2", bufs=3) as m2p,
                        tc.tile_pool(name="pd", bufs=4, space="PSUM") as pdp,
                    ):
                        for c in range(CH):
                            pD = pdp.tile([128, H], f32)
                            nc.tensor.matmul(
                                pD[:], mp_sb[:, c * 128:(c + 1) * 128],
                                w2_t[:], start=True, stop=True,
                            )
                            h2r = m2p.tile([128, H], f32, tag="h2r")
                            nc.vector.tensor_scalar(
                                h2r[:], pD[:], dis_t[:, c:c + 1], None,
                                AO.mult,
                            )
                            nc.sync.dma_start(
                                h2sh_d[c * 128:(c + 1) * 128, :], h2r[:]
                            )
                    nc.gpsimd.collective_compute(
                        "AllGather", AO.bypass, RG, [h2sh_d[:]], [tab2_d[:]]
                    )
                else:
                    # ---- readout: per-graph max ----
                    with tc.tile_pool(name="ro", bufs=1) as rop:
                        gxs = rop.tile([H, NC], f32)
                        for k in range(NC):
                            nc.vector.tensor_reduce(
                                gxs[:, k:k + 1],
                                mp_sb[:, k * GSLOT:(k + 1) * GSLOT],
                                axis=AX.X, op=AO.max,
                            )
                        nc.sync.dma_start(gxsh_d[:], gxs[:])
                    nc.gpsimd.collective_compute(
                        "AllGather", AO.bypass, RG, [gxsh_d[:]], [gxall_d[:]]
                    )

            if stage < 5:
                with tc.tile_pool(name="fb", bufs=1) as fbp:
                    fo = fbp.tile([A, B], f32)
                    nc.vector.memset(fo[:], 0.0)
                    nc.sync.dma_start(out_d[:], fo[:])

            # ---- head (redundant on every core) ----
            def bn_free(pool, psum_ap, P, nfree, gbe_t, tagp):
                """BN over the free dim + ReLU; returns activated tile."""
                pre = pool.tile([P, nfree], f32, tag=f"{tagp}pre")
                su = pool.tile([P, 1], f32, tag=f"{tagp}s1")
                nc.vector.tensor_scalar(
                    pre[:], psum_ap, 0.0, 0.0, AO.add, AO.add,
                    accum_out=su[:],
                )
                sq = pool.tile([P, nfree], f32, tag=f"{tagp}sq")
                s2 = pool.tile([P, 1], f32, tag=f"{tagp}s2")
                nc.scalar.activation(
                    sq[:], psum_ap, AF.Square, bias=zb_t[0:P, :],
                    accum_out=s2[:],
                )
                mean = pool.tile([P, 1], f32, tag=f"{tagp}mean")
                nc.vector.tensor_scalar(
                    mean[:], su[:], 1.0 / nfree, None, AO.mult
                )
                var = pool.tile([P, 1], f32, tag=f"{tagp}var")
                nc.vector.tensor_scalar(
                    var[:], s2[:], 1.0 / nfree, None, AO.mult
                )
                msq = pool.tile([P, 1], f32, tag=f"{tagp}msq")
                nc.vector.tensor_mul(msq[:], mean[:], mean[:])
                nc.vector.tensor_sub(var[:], var[:], msq[:])
                sd = pool.tile([P, 1], f32, tag=f"{tagp}sd")
                nc.scalar.activation(
                    sd[:], var[:], AF.Sqrt, bias=eps_t[0:P, :]
                )
                rinv = pool.tile([P, 1], f32, tag=f"{tagp}ri")
                nc.vector.reciprocal(rinv[:], sd[:])
                a_t = pool.tile([P, 1], f32, tag=f"{tagp}a")
                nc.vector.tensor_mul(a_t[:], rinv[:], gbe_t[:, 0:1])
                bsh = pool.tile([P, 1], f32, tag=f"{tagp}b")
                nc.vector.tensor_mul(msq[:], mean[:], a_t[:])
                nc.vector.tensor_sub(bsh[:], gbe_t[:, 1:2], msq[:])
                act = pool.tile([P, nfree], f32, tag=f"{tagp}act")
                nc.scalar.activation(
                    act[:], pre[:], AF.Relu, bias=bsh[:], scale=a_t[:]
                )
                return act

            with (
                tc.tile_pool(name="hd", bufs=1) as hd,
                tc.tile_pool(name="hp", bufs=1, space="PSUM") as hp,
            ):
              if stage >= 5:
                gxT = hd.tile([H, NC, NC], f32)
                nc.sync.dma_start(
                    gxT[:], gxall_d[:].rearrange("c f k -> f c k")
                )
                # speed encoder
                swt = hd.tile([1, 4], f32)
                nc.sync.dma_start(swt[:], swp_d[:])
                spt = hd.tile([1, B], f32)
                nc.sync.dma_start(spt[:], spt_d[:])
                sgbe = hd.tile([4, 2], f32)
                nc.sync.dma_start(sgbe[:], sgbe_d[:])
                pV = hp.tile([4, B], f32)
                nc.tensor.matmul(pV[:], swt[:], spt[:], start=True, stop=True)
                v_t = bn_free(hd, pV[:], 4, B, sgbe, "v")
                # route encoder
                cwt = hd.tile([6, 1], f32)
                nc.sync.dma_start(cwt[:], cwf_d[:])
                rimt = hd.tile([6, B * RL], f32)
                nc.sync.dma_start(rimt[:], rim_d[:])
                rgbe = hd.tile([1, 2], f32)
                nc.sync.dma_start(rgbe[:], rgbe_d[:])
                pR1 = hp.tile([1, 512], f32)
                nc.tensor.matmul(
                    pR1[:], cwt[:], rimt[:, 0:512], start=True, stop=True
                )
                pR2 = hp.tile([1, B * RL - 512], f32)
                nc.tensor.matmul(
                    pR2[:], cwt[:], rimt[:, 512:], start=True, stop=True
                )
                rcp = hd.tile([1, B * RL], f32)
                nc.vector.tensor_copy(rcp[:, 0:512], pR1[:])
                nc.vector.tensor_copy(rcp[:, 512:], pR2[:])
                rc_t = bn_free(hd, rcp[:], 1, B * RL, rgbe, "r")
                nc.sync.dma_start(rcd_d[:], rc_t[:])
                rcT = hd.tile([RL, B], f32)
                nc.sync.dma_start(
                    rcT[:], rcd_d[:].rearrange("(p g) -> p g", p=RL)
                )
                rwt = hd.tile([RL, 4], f32)
                nc.sync.dma_start(rwt[:], rw_d[:])
                rbv = hd.tile([4, 1], f32)
                nc.sync.dma_start(rbv[:], rbv_d[:])
                pW = hp.tile([4, B], f32)
                nc.tensor.matmul(pW[:], rwt[:], rcT[:], start=True, stop=True)
                r_t = hd.tile([4, B], f32)
                nc.vector.tensor_scalar(r_t[:], pW[:], rbv[:], None, AO.add)
                # concat
                cat = hd.tile([H + 8, B], f32)
                nc.vector.tensor_copy(
                    cat[0:H, :], gxT[:].rearrange("f c k -> f (c k)")
                )
                nc.sync.dma_start(cat[H:H + 4, :], v_t[:])
                nc.sync.dma_start(cat[H + 4:H + 8, :], r_t[:])
                # output MLP
                ow1t = hd.tile([H + 8, 16], f32)
                nc.sync.dma_start(ow1t[:], ow1_d[:])
                ogbe = hd.tile([16, 2], f32)
                nc.sync.dma_start(ogbe[:], ogbe_d[:])
                pH = hp.tile([16, B], f32)
                nc.tensor.matmul(pH[:], ow1t[:], cat[:], start=True, stop=True)
                o1 = bn_free(hd, pH[:], 16, B, ogbe, "o")
                ow2t = hd.tile([16, A], f32)
                nc.sync.dma_start(ow2t[:], ow2_d[:])
                ob2t = hd.tile([A, 1], f32)
                nc.sync.dma_start(ob2t[:], ob2_d[:])
                pO = hp.tile([A, B], f32)
                nc.tensor.matmul(pO[:], ow2t[:], o1[:], start=True, stop=True)
                oo = hd.tile([A, B], f32)
                nc.vector.tensor_scalar(oo[:], pO[:], ob2t[:], None, AO.add)
                nc.sync.dma_start(out_d[:], oo[:])

    nc.finalize()
    return nc


def kernel(x, edge_index, edge_weight, batch_idx, speed, route,
           W1, b1, g1, be1, W2, b2, g2, be2,
           sw, sb, sg, sbe, cw, cb, rg, rbe, rw, rb,
           ow1, ob1, og, obe, ow2, ob2):
    global LAST_EXEC_NS
    from concourse.bass_utils import run_bass_kernel_spmd

    sched, percore, _vmap = _preprocess(x, edge_index, edge_weight, batch_idx)

    key = sched["NBb"]
    if key not in _CACHE:
        _CACHE[key] = _build(sched)
    nc = _CACHE[key]

    # shared (identical) params
    eye = np.eye(128, dtype=np.float32)
    iota = np.tile(np.arange(W, dtype=np.float32)[None, :], (128, 1))
    bnp = np.stack(
        [np.asarray(g1), np.asarray(be1), np.asarray(g2), np.asarray(be2)],
        axis=1,
    ).astype(np.float32)
    route = np.asarray(route, np.float32)
    rt = route.transpose(0, 2, 1)  # [B, 2, 10]
    rtp = np.pad(rt, ((0, 0), (0, 0), (1, 1)))
    rim = np.zeros((6, B * RL), np.float32)
    cwf = np.zeros((6, 1), np.float32)
    cwa = np.asarray(cw, np.float32)
    for ch in range(2):
        for dtp in range(3):
            k = ch * 3 + dtp
            cwf[k, 0] = cwa[0, ch, dtp]
            # rim[k, pos*B + g] = rtp[g, ch, pos+dtp]
            rim[k] = rtp[:, ch, dtp:dtp + RL].T.reshape(-1)
    shared = dict(
        w1=np.asarray(W1, np.float32), w2=np.asarray(W2, np.float32),
        eye=eye, iota=iota, bnp=bnp,
        swp=np.asarray(sw, np.float32).reshape(1, 4),
        spt=np.asarray(speed, np.float32).reshape(B)[None, :],
        sgbe=np.stack([np.asarray(sg), np.asarray(sbe)], 1).astype(np.float32),
        cwf=cwf, rim=rim,
        rgbe=np.stack([np.asarray(rg), np.asarray(rbe)], 1).astype(np.float32),
        rw=np.asarray(rw, np.float32),
        rbv=np.asarray(rb, np.float32).reshape(4, 1),
        ow1=np.asarray(ow1, np.float32),
        ogbe=np.stack([np.asarray(og), np.asarray(obe)], 1).astype(np.float32),
        ow2=np.asarray(ow2, np.float32),
        ob2=np.asarray(ob2, np.float32).reshape(A, 1),
    )
    in_maps = []
    for c in range(NC):
        m = dict(shared)
        m.update(percore[c])
        in_maps.append(m)

    import os as _os

    kwargs = {}
    if _os.environ.get("KBTRACE"):
        kwargs = dict(trace=True)
        try:
            import types

            import antenv
            from trn_agent_boot.trn_boot import _ntff_profile_via_ctypes

            hook = _ntff_profile_via_ctypes("/opt/axon/libaxon_pjrt.so")
            mod = types.ModuleType("antenv.axon_hooks")
            mod.get_axon_ntff_profile_hook = lambda: hook
            mod.set_axon_ntff_profile_hook = lambda h: None
            sys.modules["antenv.axon_hooks"] = mod
            antenv.axon_hooks = mod
        except Exception as e:
            print("ntff hook injection failed:", e)
    res = run_bass_kernel_spmd(nc, in_maps, list(range(NC)), **kwargs)
    LAST_EXEC_NS = getattr(res, "exec_time_ns", None)
    out = np.asarray(res.results[0]["out"])  # [A, B]
    return np.ascontiguousarray(out.T)
